# revision 67
# baseline (speedup 1.0000x reference)
"""Trainium2 Bass kernel for nn_BasisOrbitalBackflow.

Math (reference collapses the N x N pair pooling):
    chi[b,i,mu]   = hermite_prod(ri[b,i], mu) * exp(-0.5 sigma_mu^2 |ri[b,i]|^2)
    S[b,mu]       = sum_i chi[b,i,mu]
    A[b,i,p]      = S[b,p] - chi[b,i,p]
    out[b,i,o]    = sum_{p,q} A[b,i,p] chi[b,i,q] C[p,q,o] / (N-1)

Device strategy (pure data parallel over batch, 8 cores, 256 batches each):
    * basis chi built row-major [112 partitions, 32 tiles * 20 mu] on DVE/ACT
      (Hermite polys rescaled by exact powers of two; scale folded into C)
    * PE transposes basis into a packed layout [(g:4)(mu:32)=128p, (t:8)(128)]
    * S = free-dim segment-reduce over i (DVE), A = S_bcast - B (DVE)
    * C decomposed as a rank-128 CP (ALS-refined from the truncated per-slice
      SVD init):  C[p,q,o] ~= sum_m U[p,m] V[q,m] Z[m,o]   (rel error ~1e-3,
      far under the 2e-2 gate) -> a single 128-rank chunk (NCH=1):
         out^T = Z^T @ ((U^T A^T) * (V^T B^T))
      with an exact rank-280 fallback (NCH=3) if ALS does not converge.
    * projections are fp32r TensorE matmuls with N >= 256; the elementwise
      product runs on DVE reading both PSUM operands directly
    * engine budget: Sync owns all input DMAs + per-group output DMAs,
      GpSimd warms the PE immediately and does the envelope argument,
      Scalar does biased Hermite copies / exp / PSUM drains, DVE the rest
    * output produced as out^T [14, 3584] per core; host transposes/reshapes
"""

import itertools
import numpy as np

N_MAX = 3
SDIM = 3
N_PART = 14
BATCH = 2048
NB = 20
N_CORES = 8
BC = BATCH // N_CORES          # 256 batches per core
R = BC * N_PART                # 3584 rows per core
P = 112                        # rows per tile (8 batches)
T = R // P                     # 32 tiles
G = 4                          # transposed-layout groups (8 tiles each)
TG = T // G                    # tiles per group
MS = 32                        # mu slot stride (20 real + 12 pad)

# ---------------------------------------------------------------------------
# host-side constant construction
# ---------------------------------------------------------------------------

# reference mu ordering (sorted by |n|, stable)
_NS_REF = [tuple(n) for n in sorted(
    (n for n in itertools.product(range(N_MAX + 1), repeat=SDIM) if sum(n) <= N_MAX),
    key=sum)]

# our mu ordering, chosen so the product assembly uses few, WIDE DVE ops:
# pair slots sorted by |n| so the x0-product slots 10:16 and 16:19 multiply
# CONTIGUOUS pair ranges, and the pure-copy slots pull contiguous d-slices
# of the hermite table.  pairs (n1, n2) in assembly order:
_PAIRS = [(0, 0), (1, 0), (0, 1), (2, 0), (0, 2), (1, 1), (3, 0), (1, 2), (2, 1), (0, 3)]
_NS_OURS = ([(0,) + pr for pr in _PAIRS]
            + [(1,) + _PAIRS[k] for k in range(6)]
            + [(2,) + _PAIRS[k] for k in range(3)]
            + [(3, 0, 0)])
assert sorted(_NS_OURS) == sorted(_NS_REF) and len(_NS_OURS) == NB
_PERM = np.array([_NS_REF.index(n) for n in _NS_OURS], dtype=np.int64)  # ours -> ref
_ABS_N = np.array([sum(n) for n in _NS_OURS], dtype=np.float64)


def _permuted_C(coeff):
    """Permute to our mu order, fold 1/(N-1). The 2^{|n|} Hermite rescale is
    folded into the factor ROWS after decomposition (CP-preserving), so the
    ALS error metric matches the true (unscaled) output-error metric."""
    C = np.asarray(coeff, dtype=np.float64).reshape(NB, NB, N_PART)
    return C[np.ix_(_PERM, _PERM)] / (N_PART - 1)


def _svd_terms(C):
    """Mode-3 SVD then per-slice SVD: 280 exact rank-1 terms, sorted by
    weight (descending)."""
    C3 = C.reshape(NB * NB, N_PART)
    Uo, so, Vto = np.linalg.svd(C3, full_matrices=False)
    terms = []
    for k in range(N_PART):
        Wk = (Uo[:, k] * so[k]).reshape(NB, NB)
        uu, ss, vvt = np.linalg.svd(Wk)
        for j in range(NB):
            terms.append((ss[j], uu[:, j], vvt[j], Vto[k]))
    terms.sort(key=lambda t: -t[0])
    return terms


def _als_rank128(C, iters=300, ridge=1e-12):
    """ALS CP decomposition of C (20,20,14) at rank 128, init from the
    truncated SVD terms. Returns (U, V, Z, rel_frob_err)."""
    RK = 128
    terms = _svd_terms(C)
    U = np.stack([t[1] * np.sqrt(t[0]) for t in terms[:RK]], 1)
    V = np.stack([t[2] * np.sqrt(t[0]) for t in terms[:RK]], 1)
    Z = np.stack([t[3] for t in terms[:RK]], 0).T          # (14, RK)
    T1 = C.reshape(NB, NB * N_PART)
    T2 = C.transpose(1, 0, 2).reshape(NB, NB * N_PART)
    T3 = C.transpose(2, 0, 1).reshape(N_PART, NB * NB)
    eye = ridge * np.eye(RK)
    for _ in range(iters):
        KR = (V[:, None, :] * Z[None, :, :]).reshape(NB * N_PART, RK)
        U = np.linalg.solve(KR.T @ KR + eye, KR.T @ T1.T).T
        KR = (U[:, None, :] * Z[None, :, :]).reshape(NB * N_PART, RK)
        V = np.linalg.solve(KR.T @ KR + eye, KR.T @ T2.T).T
        KR = (U[:, None, :] * V[None, :, :]).reshape(NB * NB, RK)
        Z = np.linalg.solve(KR.T @ KR + eye, KR.T @ T3.T).T
    Capx = np.einsum('pm,qm,om->pqo', U, V, Z)
    rel = np.linalg.norm(Capx - C) / max(np.linalg.norm(C), 1e-300)
    # balance per-rank factor norms for fp32 friendliness
    nu = np.linalg.norm(U, axis=0); nv = np.linalg.norm(V, axis=0)
    nz = np.linalg.norm(Z, axis=0)
    g = np.cbrt(np.maximum(nu * nv * nz, 1e-300))
    U = U * (g / np.maximum(nu, 1e-300))
    V = V * (g / np.maximum(nv, 1e-300))
    Z = Z * (g / np.maximum(nz, 1e-300))
    return U, V, Z.T, rel        # U(20,128) V(20,128) Z(128,14)


def _exact_rank280(C):
    """Exact rank-280 decomposition (fallback path, NCH=3)."""
    terms = _svd_terms(C)
    U = np.stack([t[1] * np.sqrt(t[0]) for t in terms], 1)
    V = np.stack([t[2] * np.sqrt(t[0]) for t in terms], 1)
    Z = np.stack([t[3] for t in terms], 0)
    return U, V, Z               # (20,280) (20,280) (280,14)


def _device_consts(coeff, sigma, nch, U, V, Z):
    """Pack U/V/Z/sigma into the device layouts for a given chunk count."""
    rho_pad = 128 * nch
    rho = U.shape[1]
    Upad = np.zeros((128, rho_pad)); Vpad = np.zeros((128, rho_pad))
    for g in range(G):
        Upad[32 * g:32 * g + NB, :rho] = U
        Vpad[32 * g:32 * g + NB, :rho] = V
    Zpad = np.zeros((rho_pad, N_PART)); Zpad[:rho] = Z
    # band-stacked Z weights: one [128, 128] weight per output group j with
    # Z placed in columns 32j..32j+14 (zeros elsewhere), so the four groups
    # accumulate into disjoint partition bands of a single PSUM tile
    zdev = np.zeros((128, nch * G * 128))
    for c in range(nch):
        for j in range(G):
            zdev[:, (c * G + j) * 128 + 32 * j:(c * G + j) * 128 + 32 * j + N_PART] \
                = Zpad[c * 128:(c + 1) * 128]

    sig = np.asarray(sigma, dtype=np.float64)
    s2 = -0.5 * (sig[_PERM] ** 2)
    s2c = np.broadcast_to(s2, (P, NB)).copy()

    return (Upad.astype(np.float32), Vpad.astype(np.float32),
            zdev.astype(np.float32), s2c.astype(np.float32))


_DECOMP_CACHE = {}


def _decompose(coeff, sigma):
    """Returns (nch, U, V, Zd, s2c). Prefers the rank-128 single-chunk path;
    falls back to exact rank-280 (3 chunks) if ALS does not converge."""
    key = (np.asarray(coeff).tobytes(), np.asarray(sigma).tobytes())
    hit = _DECOMP_CACHE.get(key)
    if hit is not None:
        return hit
    C = _permuted_C(coeff)
    U, V, Z, rel = _als_rank128(C)
    if rel < 4e-3:
        nch = 1
    else:
        U, V, Z = _exact_rank280(C)
        nch = 3
    # fold the 2^{|n|} Hermite rescale into the factor rows
    scale = 2.0 ** _ABS_N
    U = U * scale[:, None]
    V = V * scale[:, None]
    out = (nch,) + _device_consts(coeff, sigma, nch, U, V, Z)
    _DECOMP_CACHE[key] = out
    return out


# ---------------------------------------------------------------------------
# device program
# ---------------------------------------------------------------------------

_PROGRAMS = {}


def _build_program(nch):
    import concourse.bacc as bacc
    import concourse.tile as tile
    import concourse.mybir as mybir
    from concourse._compat import axon_active

    dt = mybir.dt
    f32 = dt.float32
    f32r = dt.float32r
    Alu = mybir.AluOpType
    ActF = mybir.ActivationFunctionType

    rho_pad = 128 * nch

    nc = bacc.Bacc(
        "TRN2",
        target_bir_lowering=False,
        debug=not axon_active(),
        num_devices=N_CORES,
    )

    x_d = nc.dram_tensor("x", [P, T * SDIM], f32, kind="ExternalInput")
    s2c_d = nc.dram_tensor("s2c", [P, NB], f32, kind="ExternalInput")
    id_d = nc.dram_tensor("ident", [P, P], f32, kind="ExternalInput")
    u_d = nc.dram_tensor("u", [128, rho_pad], f32, kind="ExternalInput")
    v_d = nc.dram_tensor("v", [128, rho_pad], f32, kind="ExternalInput")
    z_d = nc.dram_tensor("z", [128, nch * G * 128], f32, kind="ExternalInput")
    out_d = nc.dram_tensor("out_t", [128, 2 * 448], f32, kind="ExternalOutput")

    with tile.TileContext(nc) as tc:
        with (
            tc.tile_pool(name="sb", bufs=1) as sb,
            tc.tile_pool(name="ps", bufs=8, space="PSUM") as ps,
        ):
            htab = sb.tile([P, T * SDIM * 4], f32, tag="htab")
            x2 = sb.tile([P, T * SDIM], f32, tag="x2")
            x3t = sb.tile([P, T * SDIM], f32, tag="x3t")
            rho_t = sb.tile([P, T], f32, tag="rho")
            s2c = sb.tile([P, NB], f32, tag="s2c")
            ident = sb.tile([P, P], f32, tag="ident")
            u_st = sb.tile([128, rho_pad], f32, tag="u_st")
            v_st = sb.tile([128, rho_pad], f32, tag="v_st")
            z_st = sb.tile([128, nch * G * 128], f32, tag="z_st")
            u_sb = sb.tile([128, rho_pad], f32r, tag="u_sb")
            v_sb = sb.tile([128, rho_pad], f32r, tag="v_sb")
            z_sb = sb.tile([128, nch * G * 128], f32r, tag="z_sb")
            hprod = sb.tile([P, T * MS], f32, tag="hprod")
            earg = sb.tile([P, T * MS], f32, tag="earg")
            env = sb.tile([P, T * MS], f32, tag="env")
            basis = sb.tile([P, T * MS], f32, tag="basis")
            st = sb.tile([128, TG * TG], f32, tag="st")
            at_sb = sb.tile([128, TG * P], f32r, tag="at_sb")
            bt_sb = sb.tile([128, TG * P], f32r, tag="bt_sb")
            b_all = sb.tile([128, nch * G * (TG * P)], f32, tag="b_all")
            tv_sb = sb.tile([128, nch * G * (TG * P)], f32r, tag="tv_sb")
            out_sb = sb.tile([128, 2 * 448], f32, tag="out_sb")

            # ---- all input DMAs on the (otherwise idle) Sync engine -------
            h4 = htab[:].rearrange("p (n t d) -> p n t d", n=4, t=T, d=SDIM)
            nc.sync.dma_start(h4[:, 1], x_d[:].rearrange(
                "p (t d) -> p t d", t=T, d=SDIM))
            nc.sync.dma_start(s2c[:], s2c_d[:])
            nc.sync.dma_start(ident[:], id_d[:])
            nc.sync.dma_start(v_st[:], v_d[:])
            nc.sync.dma_start(u_st[:], u_d[:])
            nc.sync.dma_start(z_st[:], z_d[:])

            # ---- GpSimd: warm-up buffers FIRST -- the junk matmuls ramp the
            # PE clock governor so the full-rate grant lands on the real
            # transpose/projection phase (~12.5us in)
            wu_w = sb.tile([128, 128], dt.bfloat16, tag="wu_w")
            wu_r = sb.tile([128, 512], dt.bfloat16, tag="wu_r")
            wu_p = ps.tile([128, 512], f32, tag="pt")
            nc.gpsimd.memset(wu_w[:], 1.0)
            nc.gpsimd.memset(wu_r[:], 1.0)
            for wi in range(11):
                nc.tensor.matmul(wu_p[:], wu_w[:], wu_r[:],
                                 start=(wi == 0), stop=(wi == 10))

            # small constants on GpSimd (off the DVE critical path)
            nc.gpsimd.memset(h4[:, 0], 1.0)
            hp = hprod[:].rearrange("p (t m) -> p t m", t=T, m=MS)
            nc.gpsimd.memset(hp[:, :, 0], 1.0)

            # ---- hermite table -------------------------------------------
            # n-major layout [P, (n:4)(t:32)(d:3)]: x load and per-n writes
            # are contiguous
            x_ap = h4[:, 1]
            x2v = x2[:].rearrange("p (t d) -> p t d", t=T, d=SDIM)
            nc.vector.tensor_tensor(x2v, x_ap, x_ap, op=Alu.mult)
            nc.vector.tensor_reduce(rho_t[:], x2v, axis=mybir.AxisListType.X,
                                    op=Alu.add)
            # t3 = x^2 - 1.5 first (gates h3' on GpSimd), then h2' = x^2 - 0.5
            x3v = x3t[:].rearrange("p (t d) -> p t d", t=T, d=SDIM)
            nc.scalar.activation(x3v, x2v, ActF.Copy, bias=-1.5)
            nc.scalar.activation(h4[:, 2], x2v, ActF.Copy, bias=-0.5)
            # h3' = (x^2 - 1.5)*x on GpSimd
            nc.gpsimd.tensor_tensor(h4[:, 3], x3v, x_ap, op=Alu.mult)
            # pad mu columns of basis feed the transposes; keep them finite
            bpad = basis[:].rearrange("p (t m) -> p t m", t=T, m=MS)[:, :, NB:MS]
            nc.gpsimd.memset(bpad, 0.0)

            # envelope argument on DVE as soon as rho/s2c are there
            ea = earg[:].rearrange("p (t m) -> p t m", t=T, m=MS)[:, :, 0:NB]
            nc.vector.tensor_tensor(
                ea,
                rho_t[:].unsqueeze(-1).broadcast_to((P, T, NB)),
                s2c[:].unsqueeze(1).broadcast_to((P, T, NB)),
                op=Alu.mult)

            # ---- pair products into hprod[:, :, 0:10] --------------------
            # slots 1:3 = {H1(x1), H1(x2)},  3:5 = {H2'(x1), H2'(x2)} --
            # contiguous d-slices of the hermite table
            # on GpSimd (idle here) so they never queue behind exp on Scalar:
            # these gate the wide X-product ops and hence basis completion
            nc.gpsimd.tensor_copy(hp[:, :, 1:3], h4[:, 1, :, 1:3])
            nc.gpsimd.tensor_copy(hp[:, :, 3:5], h4[:, 2, :, 1:3])
            # slots 6, 9 = H3'(x1), H3'(x2)
            nc.gpsimd.tensor_copy(hp[:, :, 6], h4[:, 3, :, 1])
            nc.gpsimd.tensor_copy(hp[:, :, 9], h4[:, 3, :, 2])
            # cross products s5 = h1h1, s7 = h1(x1)h2'(x2), s8 = h2'(x1)h1(x2)
            x1h1 = h4[:, 1, :, 1]
            x2h1 = h4[:, 1, :, 2]
            nc.vector.tensor_tensor(hp[:, :, 5], x1h1, x2h1, op=Alu.mult)
            nc.vector.tensor_tensor(hp[:, :, 7], x1h1, h4[:, 2, :, 2],
                                    op=Alu.mult)
            nc.vector.tensor_tensor(hp[:, :, 8], h4[:, 2, :, 1], x2h1,
                                    op=Alu.mult)

            # ---- x0 products: two wide ops over contiguous pair ranges ---
            x0h1 = h4[:, 1, :, 0].unsqueeze(-1)
            nc.vector.tensor_tensor(hp[:, :, 10:16],
                                    x0h1.broadcast_to((P, T, 6)),
                                    hp[:, :, 0:6], op=Alu.mult)
            x0h2 = h4[:, 2, :, 0].unsqueeze(-1)
            nc.vector.tensor_tensor(hp[:, :, 16:19],
                                    x0h2.broadcast_to((P, T, 3)),
                                    hp[:, :, 0:3], op=Alu.mult)
            nc.gpsimd.tensor_copy(hp[:, :, 19], h4[:, 3, :, 0])

            # ---- envelope + basis (split so transposes start earlier) ----
            ev = env[:].rearrange("p (t m) -> p t m", t=T, m=MS)[:, :, 0:NB]
            bb = basis[:].rearrange("p (t m) -> p t m", t=T, m=MS)[:, :, 0:NB]
            hv = hprod[:].rearrange("p (t m) -> p t m", t=T, m=MS)[:, :, 0:NB]
            nc.scalar.activation(ev, ea, ActF.Exp)
            TH = T // 2
            nc.vector.tensor_tensor(bb[:, 0:TH], hv[:, 0:TH], ev[:, 0:TH],
                                    op=Alu.mult)
            nc.vector.tensor_tensor(bb[:, TH:T], hv[:, TH:T], ev[:, TH:T],
                                    op=Alu.mult)

            # f32r copies of the projection weights (Scalar, off-path; the
            # z copy is emitted after the drains -- only needed by Z-block)
            nc.scalar.copy(v_sb[:], v_st[:])
            nc.scalar.copy(u_sb[:], u_st[:])
            nc.scalar.copy(z_sb[:], z_st[:])

            # ---- transpose into packed layout ----------------------------
            # chunk cc covers basis cols [128cc, 128cc+128) = tiles 4cc..4cc+3
            # transposed: btp[(jb:4)(mu:32)=128p, 112] with tile t = 4c + jb.
            # 4 chunks packed per PSUM tensor at 128-col spacing; B^T copied
            # to compact SBUF [128, (k:2)(c:4)(q:112)] right after each half
            # so the V projections can start while S/A still runs on DVE.
            btps = []
            btc = bt_sb[:].rearrange("p (k c q) -> p k c q", k=2, c=4, q=P)
            for k in range(2):
                btp = ps.tile([128, 512], f32, tag="pt")
                for c in range(4):
                    nc.tensor.transpose(
                        btp[:, 128 * c:128 * c + P],
                        basis[:, 128 * (4 * k + c):128 * (4 * k + c + 1)],
                        ident[:],
                    )
                bsrc = btp[:].rearrange("p (c s) -> p c s", c=4, s=128)[:, :, 0:P]
                nc.scalar.copy(btc[:, k], bsrc)
                btps.append(btp)

            # ---- S (segment sum over i) and A = S - B per half -----------
            # entirely on GpSimd reading the compact SBUF copy of B^T, so
            # the DVE stays free for the drain/product pipeline
            stv = st[:].rearrange("p (k c b) -> p k c b", k=2, c=4, b=TG)
            atc = at_sb[:].rearrange("p (k c q) -> p k c q", k=2, c=4, q=P)
            for k in range(2):
                bsbuf = btc[:, k].bitcast(f32).rearrange("p c (b i) -> p c b i",
                                                         b=TG, i=N_PART)
                # reduce straight from the PSUM transposes: the S/A chain on
                # DVE (which gates the product chain) starts at transpose-half
                # completion instead of waiting for the Scalar bt copy
                bpsrc = btps[k][:].rearrange(
                    "p (c s) -> p c s", c=4, s=128)[:, :, 0:P].rearrange(
                    "p c (b i) -> p c b i", b=TG, i=N_PART)
                nc.vector.tensor_reduce(stv[:, k], bpsrc,
                                        axis=mybir.AxisListType.X, op=Alu.add)
                at_bi = atc[:, k].rearrange("p c (b i) -> p c b i", b=TG, i=N_PART)
                nc.gpsimd.tensor_tensor(
                    at_bi,
                    stv[:, k].unsqueeze(-1).broadcast_to((128, 4, TG, N_PART)),
                    bsbuf, op=Alu.subtract)

            # ---- rank projections + elementwise product ------------------
            # V-block first (needs only B^T), then U-block, then Z-block:
            # the PE never waits on the DVE S/A chain once it starts.
            # Drains (PSUM -> SBUF; ISA forbids two-PSUM-source DVE ops)
            # split between Scalar (g<2) and DVE (g>=2).
            tv = tv_sb[:].rearrange("p (k h c q) -> p k h c q",
                                    k=nch * G, h=2, c=4, q=P)
            ba = b_all[:].rearrange("p (k h c q) -> p k h c q",
                                    k=nch * G, h=2, c=4, q=P)
            for c in range(nch):
                for h in range(2):
                    for g in range(G):
                        rk = c * G + g
                        cs = slice(448 * h, 448 * (h + 1))
                        b_ps = ps.tile([128, 448], f32, tag="pt")
                        nc.tensor.matmul(
                            b_ps[:],
                            v_sb[32 * g:32 * g + NB, 128 * c:128 * (c + 1)],
                            bt_sb[32 * g:32 * g + NB, cs],
                            start=True, stop=True, tile_position=(32 * g, 0))
                        bv = b_ps[:].rearrange("p (j s) -> p j s", j=4, s=P)
                        if g < 3:
                            nc.scalar.copy(ba[:, rk, h], bv)
                        else:
                            nc.vector.tensor_copy(ba[:, rk, h], bv)
            # U-block: per (g, h) [128, 448] PSUM tiles (1 bank each) keep
            # the single 8-buffer rotation deep enough that no U matmul ever
            # waits on a product
            for c in range(nch):
                for h in range(2):
                    for g in range(G):
                        rk = c * G + g
                        cs = slice(448 * h, 448 * (h + 1))
                        a_ps = ps.tile([128, 448], f32, tag="pt")
                        nc.tensor.matmul(
                            a_ps[:],
                            u_sb[32 * g:32 * g + NB, 128 * c:128 * (c + 1)],
                            at_sb[32 * g:32 * g + NB, cs],
                            start=True, stop=True, tile_position=(32 * g, 0))
                        nc.vector.tensor_tensor(
                            tv_sb[:, rk * 896 + 448 * h:rk * 896 + 448 * (h + 1)],
                            a_ps[:],
                            b_all[:, rk * 896 + 448 * h:rk * 896 + 448 * (h + 1)],
                            op=Alu.mult)

            # ---- output projection: out^T = Z^T @ T ----------------------
            # band-stacked: the four j-groups accumulate into disjoint
            # 32-partition bands of ONE [128, 448] PSUM tile per half, so
            # the whole output drains with just two wide copies + two DMAs
            for h in range(2):
                o_ps = ps.tile([128, 448], f32, tag="pt")
                nmm = nch * G
                i_mm = 0
                for c in range(nch):
                    for j in range(G):
                        nc.tensor.matmul(
                            o_ps[:],
                            z_sb[:, (c * G + j) * 128:(c * G + j + 1) * 128],
                            tv_sb[:, (c * G + j) * 896 + 448 * h:
                                  (c * G + j) * 896 + 448 * (h + 1)],
                            start=(i_mm == 0), stop=(i_mm == nmm - 1))
                        i_mm += 1
                if h == 0:
                    nc.scalar.copy(out_sb[:, 0:448], o_ps[:])
                else:
                    nc.vector.tensor_copy(out_sb[:, 448:896], o_ps[:])
                nc.sync.dma_start(out_d[:, 448 * h:448 * (h + 1)],
                                  out_sb[:, 448 * h:448 * (h + 1)])

    nc.compile()
    return nc


def _get_program(nch):
    prog = _PROGRAMS.get(nch)
    if prog is None:
        prog = _PROGRAMS[nch] = _build_program(nch)
    return prog


# ---------------------------------------------------------------------------
# entry point
# ---------------------------------------------------------------------------

LAST_RESULTS = None


def kernel(ri, rij_dist=None, sigma=None, coeff=None, **_unused):
    import os
    from concourse.bass_utils import run_bass_kernel_spmd

    global LAST_RESULTS
    ri = np.ascontiguousarray(np.asarray(ri, dtype=np.float32))
    nch, U, V, Zd, s2c = _decompose(coeff, sigma)
    ident = np.eye(P, dtype=np.float32)

    nc = _get_program(nch)
    in_maps = []
    for i in range(N_CORES):
        chunk = ri[i * BC:(i + 1) * BC].reshape(T, P, SDIM)
        x = np.ascontiguousarray(chunk.transpose(1, 0, 2).reshape(P, T * SDIM))
        in_maps.append({
            "x": x, "s2c": s2c, "ident": ident,
            "u": U, "v": V, "z": Zd,
        })

    trace = bool(int(os.environ.get("BOB_TRACE", "0")))
    res = run_bass_kernel_spmd(nc, in_maps, core_ids=list(range(N_CORES)),
                               trace=trace)
    LAST_RESULTS = res

    outs = []
    for i in range(N_CORES):
        ot2 = res.results[i]["out_t"]                     # (128, 896) banded
        # partition 32j+o holds group j, orbital o; cols (h:2)(c:4)(p:112)
        ot = np.stack([ot2[32 * j:32 * j + N_PART, :] for j in range(G)],
                      axis=1).reshape(N_PART, G * 2 * 448)
        # device col order is (jb:4)(c:8)(p:112) with tile t = 4c + jb
        ot = ot.reshape(N_PART, G, 8, P).transpose(0, 2, 1, 3).reshape(N_PART, R)
        outs.append(ot.T.reshape(BC, N_PART, N_PART))
    return np.ascontiguousarray(np.concatenate(outs, axis=0), dtype=np.float32)


# revision 68
# speedup vs baseline: 1.0812x; 1.0812x over previous
"""Trainium2 Bass kernel for nn_BasisOrbitalBackflow.

Math (reference collapses the N x N pair pooling):
    chi[b,i,mu]   = hermite_prod(ri[b,i], mu) * exp(-0.5 sigma_mu^2 |ri[b,i]|^2)
    S[b,mu]       = sum_i chi[b,i,mu]
    A[b,i,p]      = S[b,p] - chi[b,i,p]
    out[b,i,o]    = sum_{p,q} A[b,i,p] chi[b,i,q] C[p,q,o] / (N-1)

Device strategy (pure data parallel over batch, 8 cores, 256 batches each):
    * basis chi built row-major [112 partitions, 32 tiles * 20 mu] on DVE/ACT
      (Hermite polys rescaled by exact powers of two; scale folded into C)
    * PE transposes basis into a packed layout [(g:4)(mu:32)=128p, (t:8)(128)]
    * S = free-dim segment-reduce over i (DVE), A = S_bcast - B (DVE)
    * C decomposed as a rank-128 CP (ALS-refined from the truncated per-slice
      SVD init):  C[p,q,o] ~= sum_m U[p,m] V[q,m] Z[m,o]   (rel error ~1e-3,
      far under the 2e-2 gate) -> a single 128-rank chunk (NCH=1):
         out^T = Z^T @ ((U^T A^T) * (V^T B^T))
      with an exact rank-280 fallback (NCH=3) if ALS does not converge.
    * projections are fp32r TensorE matmuls with N >= 256; the elementwise
      product runs on DVE reading both PSUM operands directly
    * engine budget: Sync owns all input DMAs + per-group output DMAs,
      GpSimd warms the PE immediately and does the envelope argument,
      Scalar does biased Hermite copies / exp / PSUM drains, DVE the rest
    * output produced as out^T [14, 3584] per core; host transposes/reshapes
"""

import itertools
import numpy as np

N_MAX = 3
SDIM = 3
N_PART = 14
BATCH = 2048
NB = 20
N_CORES = 8
BC = BATCH // N_CORES          # 256 batches per core
R = BC * N_PART                # 3584 rows per core
P = 112                        # rows per tile (8 batches)
T = R // P                     # 32 tiles
G = 4                          # transposed-layout groups (8 tiles each)
TG = T // G                    # tiles per group
MS = 32                        # mu slot stride (20 real + 12 pad)

# ---------------------------------------------------------------------------
# host-side constant construction
# ---------------------------------------------------------------------------

# reference mu ordering (sorted by |n|, stable)
_NS_REF = [tuple(n) for n in sorted(
    (n for n in itertools.product(range(N_MAX + 1), repeat=SDIM) if sum(n) <= N_MAX),
    key=sum)]

# our mu ordering, chosen so the product assembly uses few, WIDE DVE ops:
# pair slots sorted by |n| so the x0-product slots 10:16 and 16:19 multiply
# CONTIGUOUS pair ranges, and the pure-copy slots pull contiguous d-slices
# of the hermite table.  pairs (n1, n2) in assembly order:
_PAIRS = [(0, 0), (1, 0), (0, 1), (2, 0), (0, 2), (1, 1), (3, 0), (1, 2), (2, 1), (0, 3)]
_NS_OURS = ([(0,) + pr for pr in _PAIRS]
            + [(1,) + _PAIRS[k] for k in range(6)]
            + [(2,) + _PAIRS[k] for k in range(3)]
            + [(3, 0, 0)])
assert sorted(_NS_OURS) == sorted(_NS_REF) and len(_NS_OURS) == NB
_PERM = np.array([_NS_REF.index(n) for n in _NS_OURS], dtype=np.int64)  # ours -> ref
_ABS_N = np.array([sum(n) for n in _NS_OURS], dtype=np.float64)


def _permuted_C(coeff):
    """Permute to our mu order, fold 1/(N-1). The 2^{|n|} Hermite rescale is
    folded into the factor ROWS after decomposition (CP-preserving), so the
    ALS error metric matches the true (unscaled) output-error metric."""
    C = np.asarray(coeff, dtype=np.float64).reshape(NB, NB, N_PART)
    return C[np.ix_(_PERM, _PERM)] / (N_PART - 1)


def _svd_terms(C):
    """Mode-3 SVD then per-slice SVD: 280 exact rank-1 terms, sorted by
    weight (descending)."""
    C3 = C.reshape(NB * NB, N_PART)
    Uo, so, Vto = np.linalg.svd(C3, full_matrices=False)
    terms = []
    for k in range(N_PART):
        Wk = (Uo[:, k] * so[k]).reshape(NB, NB)
        uu, ss, vvt = np.linalg.svd(Wk)
        for j in range(NB):
            terms.append((ss[j], uu[:, j], vvt[j], Vto[k]))
    terms.sort(key=lambda t: -t[0])
    return terms


def _als_rank128(C, iters=300, ridge=1e-12):
    """ALS CP decomposition of C (20,20,14) at rank 128, init from the
    truncated SVD terms. Returns (U, V, Z, rel_frob_err)."""
    RK = 128
    terms = _svd_terms(C)
    U = np.stack([t[1] * np.sqrt(t[0]) for t in terms[:RK]], 1)
    V = np.stack([t[2] * np.sqrt(t[0]) for t in terms[:RK]], 1)
    Z = np.stack([t[3] for t in terms[:RK]], 0).T          # (14, RK)
    T1 = C.reshape(NB, NB * N_PART)
    T2 = C.transpose(1, 0, 2).reshape(NB, NB * N_PART)
    T3 = C.transpose(2, 0, 1).reshape(N_PART, NB * NB)
    eye = ridge * np.eye(RK)
    for _ in range(iters):
        KR = (V[:, None, :] * Z[None, :, :]).reshape(NB * N_PART, RK)
        U = np.linalg.solve(KR.T @ KR + eye, KR.T @ T1.T).T
        KR = (U[:, None, :] * Z[None, :, :]).reshape(NB * N_PART, RK)
        V = np.linalg.solve(KR.T @ KR + eye, KR.T @ T2.T).T
        KR = (U[:, None, :] * V[None, :, :]).reshape(NB * NB, RK)
        Z = np.linalg.solve(KR.T @ KR + eye, KR.T @ T3.T).T
    Capx = np.einsum('pm,qm,om->pqo', U, V, Z)
    rel = np.linalg.norm(Capx - C) / max(np.linalg.norm(C), 1e-300)
    # balance per-rank factor norms for fp32 friendliness
    nu = np.linalg.norm(U, axis=0); nv = np.linalg.norm(V, axis=0)
    nz = np.linalg.norm(Z, axis=0)
    g = np.cbrt(np.maximum(nu * nv * nz, 1e-300))
    U = U * (g / np.maximum(nu, 1e-300))
    V = V * (g / np.maximum(nv, 1e-300))
    Z = Z * (g / np.maximum(nz, 1e-300))
    return U, V, Z.T, rel        # U(20,128) V(20,128) Z(128,14)


def _exact_rank280(C):
    """Exact rank-280 decomposition (fallback path, NCH=3)."""
    terms = _svd_terms(C)
    U = np.stack([t[1] * np.sqrt(t[0]) for t in terms], 1)
    V = np.stack([t[2] * np.sqrt(t[0]) for t in terms], 1)
    Z = np.stack([t[3] for t in terms], 0)
    return U, V, Z               # (20,280) (20,280) (280,14)


def _device_consts(coeff, sigma, nch, U, V, Z):
    """Pack U/V/Z/sigma into the device layouts for a given chunk count."""
    rho_pad = 128 * nch
    rho = U.shape[1]
    Upad = np.zeros((128, rho_pad)); Vpad = np.zeros((128, rho_pad))
    for g in range(G):
        Upad[32 * g:32 * g + NB, :rho] = U
        Vpad[32 * g:32 * g + NB, :rho] = V
    Zpad = np.zeros((rho_pad, N_PART)); Zpad[:rho] = Z
    # band-stacked Z weights: one [128, 128] weight per output group j with
    # Z placed in columns 32j..32j+14 (zeros elsewhere), so the four groups
    # accumulate into disjoint partition bands of a single PSUM tile
    zdev = np.zeros((128, nch * G * 128))
    for c in range(nch):
        for j in range(G):
            zdev[:, (c * G + j) * 128 + 32 * j:(c * G + j) * 128 + 32 * j + N_PART] \
                = Zpad[c * 128:(c + 1) * 128]

    sig = np.asarray(sigma, dtype=np.float64)
    s2 = -0.5 * (sig[_PERM] ** 2)
    s2c = np.broadcast_to(s2, (P, NB)).copy()

    return (Upad.astype(np.float32), Vpad.astype(np.float32),
            zdev.astype(np.float32), s2c.astype(np.float32))


_DECOMP_CACHE = {}


def _decompose(coeff, sigma):
    """Returns (nch, U, V, Zd, s2c). Prefers the rank-128 single-chunk path;
    falls back to exact rank-280 (3 chunks) if ALS does not converge."""
    key = (np.asarray(coeff).tobytes(), np.asarray(sigma).tobytes())
    hit = _DECOMP_CACHE.get(key)
    if hit is not None:
        return hit
    C = _permuted_C(coeff)
    U, V, Z, rel = _als_rank128(C)
    if rel < 4e-3:
        nch = 1
    else:
        U, V, Z = _exact_rank280(C)
        nch = 3
    # fold the 2^{|n|} Hermite rescale into the factor rows
    scale = 2.0 ** _ABS_N
    U = U * scale[:, None]
    V = V * scale[:, None]
    out = (nch,) + _device_consts(coeff, sigma, nch, U, V, Z)
    _DECOMP_CACHE[key] = out
    return out


# ---------------------------------------------------------------------------
# device program
# ---------------------------------------------------------------------------

_PROGRAMS = {}


def _build_program(nch):
    import concourse.bacc as bacc
    import concourse.tile as tile
    import concourse.mybir as mybir
    from concourse._compat import axon_active

    dt = mybir.dt
    f32 = dt.float32
    f32r = dt.float32r
    Alu = mybir.AluOpType
    ActF = mybir.ActivationFunctionType

    rho_pad = 128 * nch

    nc = bacc.Bacc(
        "TRN2",
        target_bir_lowering=False,
        debug=not axon_active(),
        num_devices=N_CORES,
    )

    x_d = nc.dram_tensor("x", [P, T * SDIM], f32, kind="ExternalInput")
    s2c_d = nc.dram_tensor("s2c", [P, NB], f32, kind="ExternalInput")
    id_d = nc.dram_tensor("ident", [P, P], f32, kind="ExternalInput")
    u_d = nc.dram_tensor("u", [128, rho_pad], f32, kind="ExternalInput")
    v_d = nc.dram_tensor("v", [128, rho_pad], f32, kind="ExternalInput")
    z_d = nc.dram_tensor("z", [128, nch * G * 128], f32, kind="ExternalInput")
    out_d = nc.dram_tensor("out_t", [128, 2 * 448], f32, kind="ExternalOutput")

    with tile.TileContext(nc) as tc:
        with (
            tc.tile_pool(name="sb", bufs=1) as sb,
            tc.tile_pool(name="ps", bufs=8, space="PSUM") as ps,
        ):
            htab = sb.tile([P, T * SDIM * 4], f32, tag="htab")
            x2 = sb.tile([P, T * SDIM], f32, tag="x2")
            x3t = sb.tile([P, T * SDIM], f32, tag="x3t")
            rho_t = sb.tile([P, T], f32, tag="rho")
            s2c = sb.tile([P, NB], f32, tag="s2c")
            ident = sb.tile([P, P], f32, tag="ident")
            u_st = sb.tile([128, rho_pad], f32, tag="u_st")
            v_st = sb.tile([128, rho_pad], f32, tag="v_st")
            z_st = sb.tile([128, nch * G * 128], f32, tag="z_st")
            u_sb = sb.tile([128, rho_pad], f32r, tag="u_sb")
            v_sb = sb.tile([128, rho_pad], f32r, tag="v_sb")
            z_sb = sb.tile([128, nch * G * 128], f32r, tag="z_sb")
            hprod = sb.tile([P, T * MS], f32, tag="hprod")
            earg = sb.tile([P, T * MS], f32, tag="earg")
            env = sb.tile([P, T * MS], f32, tag="env")
            basis = sb.tile([P, T * MS], f32, tag="basis")
            st = sb.tile([128, TG * TG], f32, tag="st")
            at_sb = sb.tile([128, TG * P], f32r, tag="at_sb")
            bt_sb = sb.tile([128, TG * P], f32r, tag="bt_sb")
            b_all = sb.tile([128, nch * G * (TG * P)], f32, tag="b_all")
            tv_sb = sb.tile([128, nch * G * (TG * P)], f32r, tag="tv_sb")
            out_sb = sb.tile([128, 2 * 448], f32, tag="out_sb")

            # ---- all input DMAs on the (otherwise idle) Sync engine -------
            h4 = htab[:].rearrange("p (n t d) -> p n t d", n=4, t=T, d=SDIM)
            nc.sync.dma_start(h4[:, 1], x_d[:].rearrange(
                "p (t d) -> p t d", t=T, d=SDIM))
            nc.sync.dma_start(s2c[:], s2c_d[:])
            nc.sync.dma_start(ident[:], id_d[:])
            nc.sync.dma_start(v_st[:], v_d[:])
            nc.sync.dma_start(u_st[:], u_d[:])
            nc.sync.dma_start(z_st[:], z_d[:])

            # ---- GpSimd: warm-up buffers FIRST -- the junk matmuls ramp the
            # PE clock governor so the full-rate grant lands on the real
            # transpose/projection phase (~12.5us in)
            wu_w = sb.tile([128, 128], dt.bfloat16, tag="wu_w")
            wu_r = sb.tile([128, 512], dt.bfloat16, tag="wu_r")
            wu_p = ps.tile([128, 512], f32, tag="pt")
            nc.gpsimd.memset(wu_w[:], 1.0)
            nc.gpsimd.memset(wu_r[:], 1.0)
            for wi in range(11):
                nc.tensor.matmul(wu_p[:], wu_w[:], wu_r[:],
                                 start=(wi == 0), stop=(wi == 10))

            # small constants on GpSimd (off the DVE critical path)
            nc.gpsimd.memset(h4[:, 0], 1.0)
            hp = hprod[:].rearrange("p (t m) -> p t m", t=T, m=MS)
            nc.gpsimd.memset(hp[:, :, 0], 1.0)

            # ---- hermite table -------------------------------------------
            # n-major layout [P, (n:4)(t:32)(d:3)]: x load and per-n writes
            # are contiguous
            x_ap = h4[:, 1]
            x2v = x2[:].rearrange("p (t d) -> p t d", t=T, d=SDIM)
            nc.vector.tensor_tensor(x2v, x_ap, x_ap, op=Alu.mult)
            nc.vector.tensor_reduce(rho_t[:], x2v, axis=mybir.AxisListType.X,
                                    op=Alu.add)
            # t3 = x^2 - 1.5 first (gates h3' on GpSimd), then h2' = x^2 - 0.5
            x3v = x3t[:].rearrange("p (t d) -> p t d", t=T, d=SDIM)
            nc.scalar.activation(x3v, x2v, ActF.Copy, bias=-1.5)
            nc.scalar.activation(h4[:, 2], x2v, ActF.Copy, bias=-0.5)
            # h3' = (x^2 - 1.5)*x on GpSimd
            nc.gpsimd.tensor_tensor(h4[:, 3], x3v, x_ap, op=Alu.mult)
            # pad mu columns of basis feed the transposes; keep them finite
            bpad = basis[:].rearrange("p (t m) -> p t m", t=T, m=MS)[:, :, NB:MS]
            nc.gpsimd.memset(bpad, 0.0)

            # envelope argument on DVE as soon as rho/s2c are there
            ea = earg[:].rearrange("p (t m) -> p t m", t=T, m=MS)[:, :, 0:NB]
            nc.vector.tensor_tensor(
                ea,
                rho_t[:].unsqueeze(-1).broadcast_to((P, T, NB)),
                s2c[:].unsqueeze(1).broadcast_to((P, T, NB)),
                op=Alu.mult)

            # ---- pair products into hprod[:, :, 0:10] --------------------
            # slots 1:3 = {H1(x1), H1(x2)},  3:5 = {H2'(x1), H2'(x2)} --
            # contiguous d-slices of the hermite table
            nc.scalar.copy(hp[:, :, 1:3], h4[:, 1, :, 1:3])
            nc.scalar.copy(hp[:, :, 3:5], h4[:, 2, :, 1:3])
            # slots 6, 9 = H3'(x1), H3'(x2)
            nc.gpsimd.tensor_copy(hp[:, :, 6], h4[:, 3, :, 1])
            nc.gpsimd.tensor_copy(hp[:, :, 9], h4[:, 3, :, 2])
            # cross products s5 = h1h1, s7 = h1(x1)h2'(x2), s8 = h2'(x1)h1(x2)
            x1h1 = h4[:, 1, :, 1]
            x2h1 = h4[:, 1, :, 2]
            nc.vector.tensor_tensor(hp[:, :, 5], x1h1, x2h1, op=Alu.mult)
            nc.vector.tensor_tensor(hp[:, :, 7], x1h1, h4[:, 2, :, 2],
                                    op=Alu.mult)
            nc.vector.tensor_tensor(hp[:, :, 8], h4[:, 2, :, 1], x2h1,
                                    op=Alu.mult)

            # ---- x0 products: two wide ops over contiguous pair ranges ---
            x0h1 = h4[:, 1, :, 0].unsqueeze(-1)
            nc.vector.tensor_tensor(hp[:, :, 10:16],
                                    x0h1.broadcast_to((P, T, 6)),
                                    hp[:, :, 0:6], op=Alu.mult)
            x0h2 = h4[:, 2, :, 0].unsqueeze(-1)
            nc.vector.tensor_tensor(hp[:, :, 16:19],
                                    x0h2.broadcast_to((P, T, 3)),
                                    hp[:, :, 0:3], op=Alu.mult)
            nc.gpsimd.tensor_copy(hp[:, :, 19], h4[:, 3, :, 0])

            # ---- envelope + basis (split so transposes start earlier) ----
            ev = env[:].rearrange("p (t m) -> p t m", t=T, m=MS)[:, :, 0:NB]
            bb = basis[:].rearrange("p (t m) -> p t m", t=T, m=MS)[:, :, 0:NB]
            hv = hprod[:].rearrange("p (t m) -> p t m", t=T, m=MS)[:, :, 0:NB]
            nc.scalar.activation(ev, ea, ActF.Exp)
            TH = T // 2
            nc.vector.tensor_tensor(bb[:, 0:TH], hv[:, 0:TH], ev[:, 0:TH],
                                    op=Alu.mult)
            nc.vector.tensor_tensor(bb[:, TH:T], hv[:, TH:T], ev[:, TH:T],
                                    op=Alu.mult)

            # f32r copies of the projection weights (Scalar, off-path; the
            # z copy is emitted after the drains -- only needed by Z-block)
            nc.scalar.copy(v_sb[:], v_st[:])
            nc.scalar.copy(u_sb[:], u_st[:])
            nc.scalar.copy(z_sb[:], z_st[:])

            # ---- transpose into packed layout ----------------------------
            # chunk cc covers basis cols [128cc, 128cc+128) = tiles 4cc..4cc+3
            # transposed: btp[(jb:4)(mu:32)=128p, 112] with tile t = 4c + jb.
            # 4 chunks packed per PSUM tensor at 128-col spacing; B^T copied
            # to compact SBUF [128, (k:2)(c:4)(q:112)] right after each half
            # so the V projections can start while S/A still runs on DVE.
            btps = []
            btc = bt_sb[:].rearrange("p (k c q) -> p k c q", k=2, c=4, q=P)
            for k in range(2):
                btp = ps.tile([128, 512], f32, tag="pt")
                for c in range(4):
                    nc.tensor.transpose(
                        btp[:, 128 * c:128 * c + P],
                        basis[:, 128 * (4 * k + c):128 * (4 * k + c + 1)],
                        ident[:],
                    )
                bsrc = btp[:].rearrange("p (c s) -> p c s", c=4, s=128)[:, :, 0:P]
                nc.scalar.copy(btc[:, k], bsrc)
                btps.append(btp)

            # ---- S (segment sum over i) and A = S - B per half -----------
            # entirely on GpSimd reading the compact SBUF copy of B^T, so
            # the DVE stays free for the drain/product pipeline
            stv = st[:].rearrange("p (k c b) -> p k c b", k=2, c=4, b=TG)
            atc = at_sb[:].rearrange("p (k c q) -> p k c q", k=2, c=4, q=P)
            for k in range(2):
                bsbuf = btc[:, k].bitcast(f32).rearrange("p c (b i) -> p c b i",
                                                         b=TG, i=N_PART)
                # reduce straight from the PSUM transposes: the S/A chain on
                # DVE (which gates the product chain) starts at transpose-half
                # completion instead of waiting for the Scalar bt copy
                bpsrc = btps[k][:].rearrange(
                    "p (c s) -> p c s", c=4, s=128)[:, :, 0:P].rearrange(
                    "p c (b i) -> p c b i", b=TG, i=N_PART)
                nc.vector.tensor_reduce(stv[:, k], bpsrc,
                                        axis=mybir.AxisListType.X, op=Alu.add)
                at_bi = atc[:, k].rearrange("p c (b i) -> p c b i", b=TG, i=N_PART)
                nc.gpsimd.tensor_tensor(
                    at_bi,
                    stv[:, k].unsqueeze(-1).broadcast_to((128, 4, TG, N_PART)),
                    bsbuf, op=Alu.subtract)

            # ---- rank projections + elementwise product ------------------
            # V-block first (needs only B^T), then U-block, then Z-block:
            # the PE never waits on the DVE S/A chain once it starts.
            # Drains (PSUM -> SBUF; ISA forbids two-PSUM-source DVE ops)
            # split between Scalar (g<2) and DVE (g>=2).
            tv = tv_sb[:].rearrange("p (k h c q) -> p k h c q",
                                    k=nch * G, h=2, c=4, q=P)
            ba = b_all[:].rearrange("p (k h c q) -> p k h c q",
                                    k=nch * G, h=2, c=4, q=P)
            for c in range(nch):
                for h in range(2):
                    for g in range(G):
                        rk = c * G + g
                        cs = slice(448 * h, 448 * (h + 1))
                        b_ps = ps.tile([128, 448], f32, tag="pt")
                        nc.tensor.matmul(
                            b_ps[:],
                            v_sb[32 * g:32 * g + NB, 128 * c:128 * (c + 1)],
                            bt_sb[32 * g:32 * g + NB, cs],
                            start=True, stop=True, tile_position=(32 * g, 0))
                        bv = b_ps[:].rearrange("p (j s) -> p j s", j=4, s=P)
                        if g < 3:
                            nc.scalar.copy(ba[:, rk, h], bv)
                        else:
                            nc.vector.tensor_copy(ba[:, rk, h], bv)
            # U-block: per (g, h) [128, 448] PSUM tiles (1 bank each) keep
            # the single 8-buffer rotation deep enough that no U matmul ever
            # waits on a product
            for c in range(nch):
                for h in range(2):
                    for g in range(G):
                        rk = c * G + g
                        cs = slice(448 * h, 448 * (h + 1))
                        a_ps = ps.tile([128, 448], f32, tag="pt")
                        nc.tensor.matmul(
                            a_ps[:],
                            u_sb[32 * g:32 * g + NB, 128 * c:128 * (c + 1)],
                            at_sb[32 * g:32 * g + NB, cs],
                            start=True, stop=True, tile_position=(32 * g, 0))
                        nc.vector.tensor_tensor(
                            tv_sb[:, rk * 896 + 448 * h:rk * 896 + 448 * (h + 1)],
                            a_ps[:],
                            b_all[:, rk * 896 + 448 * h:rk * 896 + 448 * (h + 1)],
                            op=Alu.mult)

            # ---- output projection: out^T = Z^T @ T ----------------------
            # band-stacked: the four j-groups accumulate into disjoint
            # 32-partition bands of ONE [128, 448] PSUM tile per half, so
            # the whole output drains with just two wide copies + two DMAs
            for h in range(2):
                o_ps = ps.tile([128, 448], f32, tag="pt")
                nmm = nch * G
                i_mm = 0
                for c in range(nch):
                    for j in range(G):
                        nc.tensor.matmul(
                            o_ps[:],
                            z_sb[:, (c * G + j) * 128:(c * G + j + 1) * 128],
                            tv_sb[:, (c * G + j) * 896 + 448 * h:
                                  (c * G + j) * 896 + 448 * (h + 1)],
                            start=(i_mm == 0), stop=(i_mm == nmm - 1))
                        i_mm += 1
                if h == 0:
                    nc.scalar.copy(out_sb[:, 0:448], o_ps[:])
                else:
                    nc.vector.tensor_copy(out_sb[:, 448:896], o_ps[:])
                nc.sync.dma_start(out_d[:, 448 * h:448 * (h + 1)],
                                  out_sb[:, 448 * h:448 * (h + 1)])

    nc.compile()
    return nc


def _get_program(nch):
    prog = _PROGRAMS.get(nch)
    if prog is None:
        prog = _PROGRAMS[nch] = _build_program(nch)
    return prog


# ---------------------------------------------------------------------------
# entry point
# ---------------------------------------------------------------------------

LAST_RESULTS = None


def kernel(ri, rij_dist=None, sigma=None, coeff=None, **_unused):
    import os
    from concourse.bass_utils import run_bass_kernel_spmd

    global LAST_RESULTS
    ri = np.ascontiguousarray(np.asarray(ri, dtype=np.float32))
    nch, U, V, Zd, s2c = _decompose(coeff, sigma)
    ident = np.eye(P, dtype=np.float32)

    nc = _get_program(nch)
    in_maps = []
    for i in range(N_CORES):
        chunk = ri[i * BC:(i + 1) * BC].reshape(T, P, SDIM)
        x = np.ascontiguousarray(chunk.transpose(1, 0, 2).reshape(P, T * SDIM))
        in_maps.append({
            "x": x, "s2c": s2c, "ident": ident,
            "u": U, "v": V, "z": Zd,
        })

    trace = bool(int(os.environ.get("BOB_TRACE", "0")))
    res = run_bass_kernel_spmd(nc, in_maps, core_ids=list(range(N_CORES)),
                               trace=trace)
    LAST_RESULTS = res

    outs = []
    for i in range(N_CORES):
        ot2 = res.results[i]["out_t"]                     # (128, 896) banded
        # partition 32j+o holds group j, orbital o; cols (h:2)(c:4)(p:112)
        ot = np.stack([ot2[32 * j:32 * j + N_PART, :] for j in range(G)],
                      axis=1).reshape(N_PART, G * 2 * 448)
        # device col order is (jb:4)(c:8)(p:112) with tile t = 4c + jb
        ot = ot.reshape(N_PART, G, 8, P).transpose(0, 2, 1, 3).reshape(N_PART, R)
        outs.append(ot.T.reshape(BC, N_PART, N_PART))
    return np.ascontiguousarray(np.concatenate(outs, axis=0), dtype=np.float32)
